# revision 11
# baseline (speedup 1.0000x reference)
"""Duration-based length regulation (KittenTTS LengthRegulator) on 8 trn2 NeuronCores.

For each batch b (one per core): phoneme t's feature row is repeated
clamp(durations[b,t],1) times along the frame axis; frames are zero-padded to
MAX_LEN = T*15 (padding rows rely on the runner's pre-zeroed output buffers).

Per-core pipeline (batch-parallel across 8 cores):
  1. durations [128,4] load first (everything downstream of the cumsum needs
     it), then features as four per-block DMAs into [128, 8*512] tiles so
     row replication can start as soon as each block lands; the two constant
     tables ride the otherwise-idle SWDGE queue.
  2. Inclusive cumsum of clamp(dur,1) over the flattened phoneme order
     entirely on-chip: a row-wise scan (4 cols) + two PE matmuls against
     NEFF-embedded constants (upper-triangular ones / all-ones, bf16 exact
     for these small integers) accumulate the partition-dim prefix in PSUM.
  3. Offsets for all four scatter passes (s=8,4,2,1) are computed in one
     [128,16] vectorized block on DVE (before DVE touches any replication
     copy): off = exc + (dur & -(2s)), pushed OOB unless (dur & s).
  4. Row replication x8 per block by doubling copies, spread across DVE
     (blocks 0,1), ACT (block 3) and SBUF->SBUF HWDGE DMA (block 2) --
     gpsimd copies are ~4x slower than DVE, never use them.
  5. 16 indirect scatter DMAs inside a tile_critical section -- the writes
     hit disjoint output rows, so the section removes the scheduler's
     conservative WAW serialization; emissions are ordered by block
     readiness (block-major, s=8 first within a block) so SWDGE emission
     (~1.4us per call) overlaps the HBM write transfers.
Each output row is written exactly once -> DMA write traffic ~= ragged size.
"""

import sys

import numpy as np

if "/opt/trn_rl_repo" not in sys.path:
    sys.path.insert(0, "/opt/trn_rl_repo")

B, T, D = 8, 512, 512
MAX_DUR = 15
MAX_LEN = T * MAX_DUR  # 7680
P = 128
NT = T // P  # 4 feature blocks
NCOPY = 8  # replicated copies per row (binary decomposition up to 15)
SBLK = [8, 4, 2, 1]  # scatter pass block sizes
OOB = 1 << 20  # pushed past bounds_check -> descriptor silently skipped

_CACHE = {}


def _build_nc():
    import ml_dtypes
    from concourse import bass, mybir
    from concourse.bacc import Bacc
    from concourse.tile import TileContext

    f32, i32, bf16 = mybir.dt.float32, mybir.dt.int32, mybir.dt.bfloat16
    Alu = mybir.AluOpType

    nc = Bacc()
    feats = nc.declare_dram_parameter("features", [T, D], f32, isOutput=False)
    durs_mat = nc.declare_dram_parameter("durations_t", [P, NT], i32, isOutput=False)
    out = nc.declare_dram_parameter("out", [MAX_LEN, D], f32, isOutput=True)

    # NEFF-embedded constants:
    #  LO[:, 0:128]  = L, L[k, m] = 1 iff k <= m (partition-dim inclusive prefix)
    #  LO[:, 128:256] = ones (sums E_excl over partitions = block prefix)
    lo_np = np.concatenate(
        [np.triu(np.ones((P, P))), np.ones((P, P))], axis=1
    ).astype(ml_dtypes.bfloat16)
    lo_const = nc.inline_tensor(lo_np, name="lo_const")
    #  CT[:, 0:16] = -(2s) per wide column c = si*4+j; CT[:, 16:32] = s
    s_per_col = np.repeat(np.array(SBLK, np.int32), NT)  # [16]
    ct_np = np.broadcast_to(
        np.concatenate([-(2 * s_per_col), s_per_col])[None, :], (P, 2 * len(SBLK) * NT)
    ).astype(np.int32)
    ct_const = nc.inline_tensor(np.ascontiguousarray(ct_np), name="ct_const")

    NW = len(SBLK) * NT  # 16 wide columns

    with TileContext(nc) as tc:
        with tc.tile_pool(name="sbuf", bufs=1) as sb, tc.tile_pool(
            name="psum", bufs=1, space="PSUM"
        ) as pp:
            # --- loads; durations first (heads the offset critical path).
            # Feature blocks 2,3 issue from the scalar engine's HWDGE and the
            # constants ride the idle SWDGE queue, so no single DMA queue
            # serializes more than three transfers.
            dur = sb.tile([P, NT], i32, tag="dur")
            nc.sync.dma_start(out=dur[:], in_=durs_mat[:, :])
            rep = []
            for j in range(NT):
                rt = sb.tile([P, NCOPY * D], f32, tag=f"rep{j}")
                rep.append(rt)
            for j, eng in ((2, nc.scalar), (3, nc.scalar), (0, nc.sync), (1, nc.sync)):
                eng.dma_start(out=rep[j][:, 0:D], in_=feats[j * P : (j + 1) * P, :])
            lo = sb.tile([P, 2 * P], bf16, tag="lo")
            nc.gpsimd.dma_start(out=lo[:], in_=lo_const[:, :])
            ct = sb.tile([P, 2 * NW], i32, tag="ct")
            nc.gpsimd.dma_start(out=ct[:], in_=ct_const[:, :])

            # --- cumsum over flat phoneme order t = j*128 + p ------------
            nc.vector.tensor_scalar_max(out=dur[:], in0=dur[:], scalar1=1)
            dur_h = sb.tile([P, NT], bf16, tag="dur_h")
            nc.vector.tensor_copy(out=dur_h[:], in_=dur[:])
            einc = sb.tile([P, NT], bf16, tag="einc")
            nc.vector.tensor_tensor_scan(
                out=einc[:], data0=dur[:], data1=dur[:], initial=0.0,
                op0=Alu.add, op1=Alu.bypass,
            )
            eexc = sb.tile([P, NT], bf16, tag="eexc")
            nc.vector.tensor_tensor(out=eexc[:], in0=einc[:], in1=dur_h[:], op=Alu.subtract)

            ps = pp.tile([P, NT], f32, tag="ps")
            nc.tensor.matmul(ps[:], lo[:, 0:P], dur_h[:], start=True, stop=False)
            nc.tensor.matmul(ps[:], lo[:, P : 2 * P], eexc[:], start=False, stop=True)

            cum = sb.tile([P, NT], i32, tag="cum")
            nc.vector.tensor_copy(out=cum[:], in_=ps[:])
            exc = sb.tile([P, NT], i32, tag="exc")
            nc.vector.tensor_tensor(out=exc[:], in0=cum[:], in1=dur[:], op=Alu.subtract)

            # --- widen dur/exc to [128, 16] (4 copies along s-passes) ----
            dur16 = sb.tile([P, NW], i32, tag="dur16")
            exc16 = sb.tile([P, NW], i32, tag="exc16")
            nc.vector.tensor_copy(out=dur16[:, 0:NT], in_=dur[:])
            nc.vector.tensor_copy(out=dur16[:, NT : 2 * NT], in_=dur[:])
            nc.vector.tensor_copy(out=dur16[:, 2 * NT : 4 * NT], in_=dur16[:, 0 : 2 * NT])
            nc.vector.tensor_copy(out=exc16[:, 0:NT], in_=exc[:])
            nc.vector.tensor_copy(out=exc16[:, NT : 2 * NT], in_=exc[:])
            nc.vector.tensor_copy(out=exc16[:, 2 * NT : 4 * NT], in_=exc16[:, 0 : 2 * NT])

            # --- scatter offsets, all passes at once ---------------------
            offs = sb.tile([P, NW], i32, tag="offs")
            msk = sb.tile([P, NW], i32, tag="msk")
            nc.vector.tensor_tensor(out=offs[:], in0=dur16[:], in1=ct[:, 0:NW], op=Alu.bitwise_and)
            nc.vector.tensor_tensor(out=offs[:], in0=offs[:], in1=exc16[:], op=Alu.add)
            nc.vector.tensor_tensor(out=msk[:], in0=dur16[:], in1=ct[:, NW : 2 * NW], op=Alu.bitwise_and)
            nc.vector.tensor_scalar(
                out=msk[:], in0=msk[:], scalar1=0, scalar2=OOB, op0=Alu.is_equal, op1=Alu.mult
            )
            nc.vector.tensor_tensor(out=offs[:], in0=offs[:], in1=msk[:], op=Alu.add)

            breg = nc.gpsimd.to_reg(MAX_LEN - 1)
            sc_sem = nc.alloc_semaphore("scatter_sem")
            n_sc = 0

            def scatter(j, s_, last=False):
                # One critical section per (pass, block): disjoint writes ->
                # no inter-DMA completion waits, and because the dependency
                # tracker only sees writers emitted SO FAR, a section waits
                # just for the copies its pass actually reads (s=1 only needs
                # the raw feature load). no_gpsimd_drain keeps a section's
                # exit from waiting for its own transfers; the final wait_ge
                # gates kernel teardown on all 16 completions.
                nonlocal n_sc
                si = SBLK.index(s_)
                with tc.tile_critical(no_gpsimd_drain=True):
                    c = si * NT + j
                    nc.gpsimd.indirect_dma_start(
                        out=out[:, :],
                        out_offset=bass.IndirectOffsetOnAxis(
                            ap=offs[:, c : c + 1], axis=0
                        ),
                        in_=rep[j][:, 0 : s_ * D],
                        in_offset=None,
                        bounds_check=breg,
                        oob_is_err=False,
                    ).then_inc(sc_sem, 16)
                    n_sc += 1
                    if last:
                        nc.gpsimd.wait_ge(sc_sem, n_sc * 16)

            # --- row replication x8 per block, interleaved with the scatter
            # sections in readiness order. DVE: plain doubling copies for
            # blocks 0,1 (fenced behind the offset chain via a scheduling
            # wait so the greedy per-engine scheduler cannot slot a long
            # copy into an offset-chain semaphore stall); ACT: blocks 2,3
            # as single stride-0 broadcast-read ops.
            def dve_copy(j, w):
                with tc.tile_wait_until(0.03):
                    nc.vector.tensor_copy(
                        out=rep[j][:, w * D : 2 * w * D], in_=rep[j][:, 0 : w * D]
                    )

            def act_repl(j):
                src = rep[j][:, 0:D].rearrange("p (x d) -> p x d", x=1).to_broadcast(
                    [P, NCOPY - 1, D]
                )
                dst = rep[j][:, D : NCOPY * D].rearrange("p (x d) -> p x d", d=D)
                nc.scalar.copy(out=dst, in_=src)

            scatter(2, 1)
            scatter(0, 1)
            dve_copy(0, 1)
            scatter(0, 2)
            scatter(3, 1)
            scatter(1, 1)
            dve_copy(0, 2)
            scatter(0, 4)
            act_repl(2)
            scatter(2, 2)
            scatter(2, 4)
            scatter(2, 8)
            dve_copy(0, 4)
            scatter(0, 8)
            dve_copy(1, 1)
            scatter(1, 2)
            dve_copy(1, 2)
            scatter(1, 4)
            dve_copy(1, 4)
            scatter(1, 8)
            act_repl(3)
            scatter(3, 2)
            scatter(3, 4)
            scatter(3, 8, last=True)

    nc.compile()
    return nc


def _get_nc():
    if "nc" not in _CACHE:
        _CACHE["nc"] = _build_nc()
    return _CACHE["nc"]


def _run(features, durations, trace=False):
    """features (B,T,D) f32, durations (B,T) i32 -> (out (B,MAX_LEN,D) f32, BassKernelResults)."""
    from concourse.bass_utils import run_bass_kernel_spmd

    nc = _get_nc()
    in_maps = []
    for b in range(B):
        dmat = np.ascontiguousarray(durations[b].reshape(NT, P).T)  # [P, NT]
        in_maps.append(
            {
                "features": np.ascontiguousarray(features[b]),
                "durations_t": dmat,
            }
        )
    kwargs = {}
    if trace:
        kwargs = dict(trace=True, trace_cores=list(range(B)), stitch_traces=False)
    res = run_bass_kernel_spmd(nc, in_maps, core_ids=list(range(B)), **kwargs)
    outs = np.stack([res.results[b]["out"] for b in range(B)])
    return outs.astype(np.float32, copy=False), res


def kernel(features, durations):
    features = np.asarray(features, dtype=np.float32)
    durations = np.asarray(durations, dtype=np.int32)
    outs, _ = _run(features, durations, trace=False)
    return outs


if __name__ == "__main__":
    feats = np.random.randn(B, T, D).astype(np.float32)
    durs = np.random.randint(0, 16, size=(B, T)).astype(np.int32)
    out = kernel(feats, durs)
    print("out", out.shape, out.dtype)


# revision 12
# speedup vs baseline: 1.1272x; 1.1272x over previous
"""Duration-based length regulation (KittenTTS LengthRegulator) on 8 trn2 NeuronCores.

For each batch b (one per core): phoneme t's feature row is repeated
clamp(durations[b,t],1) times along the frame axis; frames are zero-padded to
MAX_LEN = T*15 (padding rows rely on the runner's pre-zeroed output buffers).

Per-core pipeline (batch-parallel across 8 cores):
  1. Loads split across three DMA issuers (sync, scalar-HWDGE) so no queue
     serializes more than four transfers: durations first (heads the offset
     critical path), then the four feature blocks and two NEFF-embedded
     constant tables.
  2. Inclusive cumsum of clamp(dur,1) over the flattened phoneme order
     entirely on-chip: a row-wise scan (4 cols) + two PE matmuls against
     embedded constants (upper-triangular ones / all-ones, bf16 exact for
     these small integers) accumulate the partition-dim prefix in PSUM.
  3. Offsets for all four scatter passes (s=8,4,2,1) in one [128,16]
     vectorized block on DVE: off = exc + (dur & -(2s)), pushed OOB
     (>= 1<<20) unless (dur & s). DVE replication copies carry a scheduling
     fence (tile_wait_until) so the greedy per-engine scheduler cannot slot
     a long copy into an offset-chain semaphore stall.
  4. Row replication x8: DVE doubling copies for blocks 0,1; single
     stride-0 broadcast-read ops on ACT for blocks 2,3.
  5. 16 indirect scatter DMAs (SWDGE descriptor emission is ~1.4us each and
     one-offset-per-partition is a firmware limit, so 16 is minimal for the
     binary decomposition). Each block writes ITS OWN output tensor, so the
     scheduler's WAW completion chains only link same-block passes, which a
     round-robin emission order spaces ~5.6us apart - the Pool engine never
     stalls. The host adds the four pre-zeroed disjoint buffers.
Each output row is written exactly once -> DMA write traffic ~= ragged size.
"""

import sys

import numpy as np

if "/opt/trn_rl_repo" not in sys.path:
    sys.path.insert(0, "/opt/trn_rl_repo")

B, T, D = 8, 512, 512
MAX_DUR = 15
MAX_LEN = T * MAX_DUR  # 7680
P = 128
NT = T // P  # 4 feature blocks
NCOPY = 8  # replicated copies per row (binary decomposition up to 15)
SBLK = [8, 4, 2, 1]  # scatter pass block sizes
OOB = 1 << 20  # pushed past bounds_check -> descriptor silently skipped

_CACHE = {}


def _build_nc():
    import ml_dtypes
    from concourse import bass, mybir
    from concourse.bacc import Bacc
    from concourse.tile import TileContext

    f32, i32, bf16 = mybir.dt.float32, mybir.dt.int32, mybir.dt.bfloat16
    Alu = mybir.AluOpType

    nc = Bacc()
    feats = nc.declare_dram_parameter("features", [T, D], f32, isOutput=False)
    durs_mat = nc.declare_dram_parameter("durations_t", [P, NT], i32, isOutput=False)
    outs = [
        nc.declare_dram_parameter(f"out{j}", [MAX_LEN, D], f32, isOutput=True)
        for j in range(NT)
    ]

    # NEFF-embedded constants:
    #  LO[:, 0:128]  = L, L[k, m] = 1 iff k <= m (partition-dim inclusive prefix)
    #  LO[:, 128:256] = ones (sums E_excl over partitions = block prefix)
    lo_np = np.concatenate(
        [np.triu(np.ones((P, P))), np.ones((P, P))], axis=1
    ).astype(ml_dtypes.bfloat16)
    lo_const = nc.inline_tensor(lo_np, name="lo_const")
    #  CT[:, 0:16] = -(2s) per wide column c = si*4+j; CT[:, 16:32] = s
    s_per_col = np.repeat(np.array(SBLK, np.int32), NT)  # [16]
    ct_np = np.broadcast_to(
        np.concatenate([-(2 * s_per_col), s_per_col])[None, :], (P, 2 * len(SBLK) * NT)
    ).astype(np.int32)
    ct_const = nc.inline_tensor(np.ascontiguousarray(ct_np), name="ct_const")

    NW = len(SBLK) * NT  # 16 wide columns

    with TileContext(nc) as tc:
        with tc.tile_pool(name="sbuf", bufs=1) as sb, tc.tile_pool(
            name="psum", bufs=1, space="PSUM"
        ) as pp:
            # --- loads --------------------------------------------------
            dur = sb.tile([P, NT], i32, tag="dur")
            nc.sync.dma_start(out=dur[:], in_=durs_mat[:, :])
            lo = sb.tile([P, 2 * P], bf16, tag="lo")
            nc.scalar.dma_start(out=lo[:], in_=lo_const[:, :])
            ct = sb.tile([P, 2 * NW], i32, tag="ct")
            nc.sync.dma_start(out=ct[:], in_=ct_const[:, :])
            rep = []
            for j in range(NT):
                rt = sb.tile([P, NCOPY * D], f32, tag=f"rep{j}")
                rep.append(rt)
            for j, eng in ((2, nc.scalar), (3, nc.scalar), (0, nc.sync), (1, nc.sync)):
                eng.dma_start(out=rep[j][:, 0:D], in_=feats[j * P : (j + 1) * P, :])

            # --- cumsum over flat phoneme order t = j*128 + p -----------
            nc.vector.tensor_scalar_max(out=dur[:], in0=dur[:], scalar1=1)
            dur_h = sb.tile([P, NT], bf16, tag="dur_h")
            nc.vector.tensor_copy(out=dur_h[:], in_=dur[:])
            einc = sb.tile([P, NT], bf16, tag="einc")
            nc.vector.tensor_tensor_scan(
                out=einc[:], data0=dur[:], data1=dur[:], initial=0.0,
                op0=Alu.add, op1=Alu.bypass,
            )
            eexc = sb.tile([P, NT], bf16, tag="eexc")
            nc.vector.tensor_tensor(out=eexc[:], in0=einc[:], in1=dur_h[:], op=Alu.subtract)

            ps = pp.tile([P, NT], f32, tag="ps")
            nc.tensor.matmul(ps[:], lo[:, 0:P], dur_h[:], start=True, stop=False)
            nc.tensor.matmul(ps[:], lo[:, P : 2 * P], eexc[:], start=False, stop=True)

            cum = sb.tile([P, NT], i32, tag="cum")
            nc.vector.tensor_copy(out=cum[:], in_=ps[:])
            exc = sb.tile([P, NT], i32, tag="exc")
            nc.vector.tensor_tensor(out=exc[:], in0=cum[:], in1=dur[:], op=Alu.subtract)

            # --- widen dur/exc to [128, 16] (4 copies along s-passes) ---
            dur16 = sb.tile([P, NW], i32, tag="dur16")
            exc16 = sb.tile([P, NW], i32, tag="exc16")
            nc.vector.tensor_copy(out=dur16[:, 0:NT], in_=dur[:])
            nc.vector.tensor_copy(out=dur16[:, NT : 2 * NT], in_=dur[:])
            nc.vector.tensor_copy(out=dur16[:, 2 * NT : 4 * NT], in_=dur16[:, 0 : 2 * NT])
            nc.vector.tensor_copy(out=exc16[:, 0:NT], in_=exc[:])
            nc.vector.tensor_copy(out=exc16[:, NT : 2 * NT], in_=exc[:])
            nc.vector.tensor_copy(out=exc16[:, 2 * NT : 4 * NT], in_=exc16[:, 0 : 2 * NT])

            # --- scatter offsets, all passes at once --------------------
            offs = sb.tile([P, NW], i32, tag="offs")
            msk = sb.tile([P, NW], i32, tag="msk")
            nc.vector.tensor_tensor(out=offs[:], in0=dur16[:], in1=ct[:, 0:NW], op=Alu.bitwise_and)
            nc.vector.tensor_tensor(out=offs[:], in0=offs[:], in1=exc16[:], op=Alu.add)
            nc.vector.tensor_tensor(out=msk[:], in0=dur16[:], in1=ct[:, NW : 2 * NW], op=Alu.bitwise_and)
            nc.vector.tensor_scalar(
                out=msk[:], in0=msk[:], scalar1=0, scalar2=OOB, op0=Alu.is_equal, op1=Alu.mult
            )
            nc.vector.tensor_tensor(out=offs[:], in0=offs[:], in1=msk[:], op=Alu.add)

            # --- row replication ----------------------------------------
            def dve_copy(j, w):
                with tc.tile_wait_until(0.012):
                    nc.vector.tensor_copy(
                        out=rep[j][:, w * D : 2 * w * D], in_=rep[j][:, 0 : w * D]
                    )

            def act_repl(j):
                src = rep[j][:, 0:D].rearrange("p (x d) -> p x d", x=1).to_broadcast(
                    [P, NCOPY - 1, D]
                )
                dst = rep[j][:, D : NCOPY * D].rearrange("p (x d) -> p x d", d=D)
                nc.scalar.copy(out=dst, in_=src)

            act_repl(2)
            for w in (1, 2, 4):
                dve_copy(0, w)
            act_repl(3)
            for w in (1, 2, 4):
                dve_copy(1, w)

            breg = nc.gpsimd.to_reg(MAX_LEN - 1)

            def scatter(j, s_):
                si = SBLK.index(s_)
                c = si * NT + j
                nc.gpsimd.indirect_dma_start(
                    out=outs[j][:, :],
                    out_offset=bass.IndirectOffsetOnAxis(ap=offs[:, c : c + 1], axis=0),
                    in_=rep[j][:, 0 : s_ * D],
                    in_offset=None,
                    bounds_check=breg,
                    oob_is_err=False,
                )

            # Round-robin over blocks so same-output WAW chains get ~5.6us
            # of slack; s=1 first (needs only the raw load), s=4 last so the
            # final transfers are small (short completion tail).
            for s_ in (1, 2, 8, 4):
                for j in (2, 0, 3, 1):
                    scatter(j, s_)

    nc.compile()
    return nc


def _get_nc():
    if "nc" not in _CACHE:
        _CACHE["nc"] = _build_nc()
    return _CACHE["nc"]


def _run(features, durations, trace=False):
    """features (B,T,D) f32, durations (B,T) i32 -> (out (B,MAX_LEN,D) f32, BassKernelResults)."""
    from concourse.bass_utils import run_bass_kernel_spmd

    nc = _get_nc()
    in_maps = []
    for b in range(B):
        dmat = np.ascontiguousarray(durations[b].reshape(NT, P).T)  # [P, NT]
        in_maps.append(
            {
                "features": np.ascontiguousarray(features[b]),
                "durations_t": dmat,
            }
        )
    kwargs = {}
    if trace:
        kwargs = dict(trace=True, trace_cores=list(range(B)), stitch_traces=False)
    res = run_bass_kernel_spmd(nc, in_maps, core_ids=list(range(B)), **kwargs)
    # per-block outputs write disjoint rows of pre-zeroed buffers: sum merges
    outs = np.stack(
        [sum(res.results[b][f"out{j}"] for j in range(NT)) for b in range(B)]
    )
    return outs.astype(np.float32, copy=False), res


def kernel(features, durations):
    features = np.asarray(features, dtype=np.float32)
    durations = np.asarray(durations, dtype=np.int32)
    outs, _ = _run(features, durations, trace=False)
    return outs


if __name__ == "__main__":
    feats = np.random.randn(B, T, D).astype(np.float32)
    durs = np.random.randint(0, 16, size=(B, T)).astype(np.int32)
    out = kernel(feats, durs)
    print("out", out.shape, out.dtype)


# revision 14
# speedup vs baseline: 1.2517x; 1.1105x over previous
"""Duration-based length regulation (KittenTTS LengthRegulator) on 8 trn2 NeuronCores.

For each batch b (one per core): phoneme t's feature row is repeated
clamp(durations[b,t],1) times along the frame axis; frames are zero-padded to
MAX_LEN = T*15 (padding rows rely on the runner's pre-zeroed output buffers).

Phonemes map to (partition, block) as t = 4p + j, so ONE feature DMA lands
all 512 rows with contiguous 8KB-per-partition descriptors (3x the delivery
rate of row-per-partition 2KB descriptors).

Per-core pipeline (batch-parallel across 8 cores):
  1. Loads: durations (sync, first - heads the offset critical path),
     features in one DMA into a [128, 4*512] landing tile (sync), constant
     tables on the scalar engine's HWDGE queue.
  2. Inclusive cumsum of clamp(dur,1) over flat order t = 4p+j: free-dim
     row scan + ONE PE matmul (strict-lower-triangular ones, bf16 exact for
     these small integers) for the partition-dim prefix of row sums.
  3. Offsets for all four scatter passes (s=8,4,2,1) in one [128,16]
     vectorized block on DVE: off = exc + (dur & -(2s)), pushed OOB
     (>= 1<<20) unless (dur & s). DVE replication copies carry a scheduling
     fence (tile_wait_until) so the greedy per-engine scheduler cannot slot
     a long copy into an offset-chain semaphore stall.
  4. Row replication x8 into per-block [128, 8*512] tiles (kept at 16KB per
     partition - bigger tiles lose the DVE 4x perf mode): DVE doubling
     copies for blocks 0,1 (+ block 3 tail), ACT stride-0 broadcast-read
     ops for blocks 2,3.
  5. 16 indirect scatter DMAs (SWDGE emission is ~1.4us each and
     one-offset-per-partition is a firmware limit, so 16 is minimal for the
     binary decomposition) inside four per-block tile_critical sections in
     block-readiness order: the writes hit disjoint output rows, so the
     sections remove the scheduler's conservative WAW completion chains;
     no_gpsimd_drain keeps a section's exit from waiting for its own
     transfers; the final wait_ge gates teardown on all 16 completions.
Each output row is written exactly once -> DMA write traffic ~= ragged size.
"""

import sys

import numpy as np

if "/opt/trn_rl_repo" not in sys.path:
    sys.path.insert(0, "/opt/trn_rl_repo")

B, T, D = 8, 512, 512
MAX_DUR = 15
MAX_LEN = T * MAX_DUR  # 7680
P = 128
NT = T // P  # 4 feature blocks
NCOPY = 8  # replicated copies per row (binary decomposition up to 15)
SBLK = [8, 4, 2, 1]  # scatter pass block sizes
OOB = 1 << 20  # pushed past bounds_check -> descriptor silently skipped

_CACHE = {}


def _build_nc():
    import ml_dtypes
    from concourse import bass, mybir
    from concourse.bacc import Bacc
    from concourse.tile import TileContext

    f32, i32, bf16 = mybir.dt.float32, mybir.dt.int32, mybir.dt.bfloat16
    Alu = mybir.AluOpType

    nc = Bacc()
    feats = nc.declare_dram_parameter("features", [T, D], f32, isOutput=False)
    durs_mat = nc.declare_dram_parameter("durations_t", [P, NT], i32, isOutput=False)
    out = nc.declare_dram_parameter("out", [MAX_LEN, D], f32, isOutput=True)

    # NEFF-embedded constants:
    #  LO[:, 0:128] = Lstrict, L[k, m] = 1 iff k < m (exclusive partition prefix)
    lo_np = (np.arange(P)[:, None] < np.arange(P)[None, :]).astype(ml_dtypes.bfloat16)
    lo_const = nc.inline_tensor(np.ascontiguousarray(lo_np), name="lo_const")
    #  CT[:, 0:16] = -(2s) per wide column c = si*4+j; CT[:, 16:32] = s
    s_per_col = np.repeat(np.array(SBLK, np.int32), NT)  # [16]
    ct_np = np.broadcast_to(
        np.concatenate([-(2 * s_per_col), s_per_col])[None, :], (P, 2 * len(SBLK) * NT)
    ).astype(np.int32)
    ct_const = nc.inline_tensor(np.ascontiguousarray(ct_np), name="ct_const")

    NW = len(SBLK) * NT  # 16 wide columns

    with TileContext(nc) as tc:
        with tc.tile_pool(name="sbuf", bufs=1) as sb, tc.tile_pool(
            name="psum", bufs=1, space="PSUM"
        ) as pp:
            # --- loads --------------------------------------------------
            dur = sb.tile([P, NT], i32, tag="dur")
            nc.sync.dma_start(out=dur[:], in_=durs_mat[:, :])
            land = sb.tile([P, NT * D], f32, tag="land")
            nc.sync.dma_start(
                out=land[:], in_=feats[:, :].rearrange("(p j) d -> p (j d)", j=NT)
            )
            lo = sb.tile([P, P], bf16, tag="lo")
            nc.scalar.dma_start(out=lo[:], in_=lo_const[:, :])
            ct = sb.tile([P, 2 * NW], i32, tag="ct")
            nc.scalar.dma_start(out=ct[:], in_=ct_const[:, :])
            rep = []
            for j in range(NT):
                rt = sb.tile([P, NCOPY * D], f32, tag=f"rep{j}")
                rep.append(rt)

            # --- cumsum over flat phoneme order t = 4p + j --------------
            nc.vector.tensor_scalar_max(out=dur[:], in0=dur[:], scalar1=1)
            einc = sb.tile([P, NT], i32, tag="einc")
            nc.vector.tensor_tensor_scan(
                out=einc[:], data0=dur[:], data1=dur[:], initial=0.0,
                op0=Alu.add, op1=Alu.bypass,
            )
            rs_h = sb.tile([P, 1], bf16, tag="rs_h")
            nc.vector.tensor_copy(out=rs_h[:], in_=einc[:, NT - 1 : NT])

            ps = pp.tile([P, 1], f32, tag="ps")
            nc.tensor.matmul(ps[:], lo[:, :], rs_h[:], start=True, stop=True)
            pfx = sb.tile([P, 1], i32, tag="pfx")
            nc.vector.tensor_copy(out=pfx[:], in_=ps[:])

            cum = sb.tile([P, NT], i32, tag="cum")
            nc.vector.tensor_tensor(
                out=cum[:], in0=einc[:], in1=pfx[:].to_broadcast([P, NT]), op=Alu.add
            )
            exc = sb.tile([P, NT], i32, tag="exc")
            nc.vector.tensor_tensor(out=exc[:], in0=cum[:], in1=dur[:], op=Alu.subtract)

            # --- widen dur/exc to [128, 16] (4 copies along s-passes) ---
            dur16 = sb.tile([P, NW], i32, tag="dur16")
            exc16 = sb.tile([P, NW], i32, tag="exc16")
            nc.vector.tensor_copy(out=dur16[:, 0:NT], in_=dur[:])
            nc.vector.tensor_copy(out=dur16[:, NT : 2 * NT], in_=dur[:])
            nc.vector.tensor_copy(out=dur16[:, 2 * NT : 4 * NT], in_=dur16[:, 0 : 2 * NT])
            nc.vector.tensor_copy(out=exc16[:, 0:NT], in_=exc[:])
            nc.vector.tensor_copy(out=exc16[:, NT : 2 * NT], in_=exc[:])
            nc.vector.tensor_copy(out=exc16[:, 2 * NT : 4 * NT], in_=exc16[:, 0 : 2 * NT])

            # --- scatter offsets, all passes at once --------------------
            offs = sb.tile([P, NW], i32, tag="offs")
            msk = sb.tile([P, NW], i32, tag="msk")
            nc.vector.tensor_tensor(out=offs[:], in0=dur16[:], in1=ct[:, 0:NW], op=Alu.bitwise_and)
            nc.vector.tensor_tensor(out=offs[:], in0=offs[:], in1=exc16[:], op=Alu.add)
            nc.vector.tensor_tensor(out=msk[:], in0=dur16[:], in1=ct[:, NW : 2 * NW], op=Alu.bitwise_and)
            nc.vector.tensor_scalar(
                out=msk[:], in0=msk[:], scalar1=0, scalar2=OOB, op0=Alu.is_equal, op1=Alu.mult
            )
            nc.vector.tensor_tensor(out=offs[:], in0=offs[:], in1=msk[:], op=Alu.add)

            # --- row replication ----------------------------------------
            def dve_block(j):
                with tc.tile_wait_until(0.012):
                    nc.vector.tensor_copy(out=rep[j][:, 0:D], in_=land[:, j * D : (j + 1) * D])
                for w in (1, 2, 4):
                    with tc.tile_wait_until(0.012):
                        nc.vector.tensor_copy(
                            out=rep[j][:, w * D : 2 * w * D], in_=rep[j][:, 0 : w * D]
                        )

            def act_block(j):
                nc.scalar.copy(out=rep[j][:, 0:D], in_=land[:, j * D : (j + 1) * D])
                src = rep[j][:, 0:D].rearrange("p (x d) -> p x d", x=1).to_broadcast(
                    [P, NCOPY - 1, D]
                )
                dst = rep[j][:, D : NCOPY * D].rearrange("p (x d) -> p x d", d=D)
                nc.scalar.copy(out=dst, in_=src)

            dve_block(0)
            act_block(2)
            dve_block(1)
            act_block(3)

            breg = nc.gpsimd.to_reg(MAX_LEN - 1)
            sc_sem = nc.alloc_semaphore("scatter_sem")
            n_sc = 0

            # --- scatters: four per-block critical sections -------------
            for bi, j in enumerate((0, 2, 1, 3)):
                last = bi == NT - 1
                with tc.tile_critical(no_gpsimd_drain=True):
                    for s_ in SBLK:
                        si = SBLK.index(s_)
                        c = si * NT + j
                        nc.gpsimd.indirect_dma_start(
                            out=out[:, :],
                            out_offset=bass.IndirectOffsetOnAxis(
                                ap=offs[:, c : c + 1], axis=0
                            ),
                            in_=rep[j][:, 0 : s_ * D],
                            in_offset=None,
                            bounds_check=breg,
                            oob_is_err=False,
                        ).then_inc(sc_sem, 16)
                        n_sc += 1
                    if last:
                        nc.gpsimd.wait_ge(sc_sem, n_sc * 16)

    nc.compile()
    return nc


def _get_nc():
    if "nc" not in _CACHE:
        _CACHE["nc"] = _build_nc()
    return _CACHE["nc"]


def _run(features, durations, trace=False):
    """features (B,T,D) f32, durations (B,T) i32 -> (out (B,MAX_LEN,D) f32, BassKernelResults)."""
    from concourse.bass_utils import run_bass_kernel_spmd

    nc = _get_nc()
    in_maps = []
    for b in range(B):
        dmat = np.ascontiguousarray(durations[b].reshape(P, NT))  # [P, NT], t = 4p+j
        in_maps.append(
            {
                "features": np.ascontiguousarray(features[b]),
                "durations_t": dmat,
            }
        )
    kwargs = {}
    if trace:
        kwargs = dict(trace=True, trace_cores=list(range(B)), stitch_traces=False)
    res = run_bass_kernel_spmd(nc, in_maps, core_ids=list(range(B)), **kwargs)
    outs = np.stack([res.results[b]["out"] for b in range(B)])
    return outs.astype(np.float32, copy=False), res


def kernel(features, durations):
    features = np.asarray(features, dtype=np.float32)
    durations = np.asarray(durations, dtype=np.int32)
    outs, _ = _run(features, durations, trace=False)
    return outs


if __name__ == "__main__":
    feats = np.random.randn(B, T, D).astype(np.float32)
    durs = np.random.randint(0, 16, size=(B, T)).astype(np.int32)
    out = kernel(feats, durs)
    print("out", out.shape, out.dtype)
